# revision 3
# baseline (speedup 1.0000x reference)
"""BitAttention TRN2 kernel v2: 8-core SPMD (DP batch x TP kv-heads),
token-sharded x-quantization with AllGather, transposed-scores attention.

Core r: batch b = r//4, kv-head kh = r%4, x/out token-quarter qx = r%4.

Math (forward-equivalent to the reference):
  - linear_bit exact-integer path: int8 activations (MAGIC round) and ternary
    weights ({-1,0,1} via two fused compares) are exact in bf16; projections
    run as exact-integer bf16 matmuls, dequant scales applied post-PSUM.
  - The reference einsum sums the query-head group axis -> group-sum ternary
    w_q rows (ints in [-4,4], exact bf16).
  - Both /sqrt(HD) scalings fold into one *(1/128) on q's dequant scale.
  - Scores are tiny (|s| < ~1): softmax needs no max subtraction. Scores are
    computed TRANSPOSED (ST[key, tok] = K @ Q^T) so P^T feeds P@V directly
    with no transpose; Z comes from a ones-column appended to V.
  - Weights stream from HBM once: f32 chunks are held in SBUF between the
    abs-mean pass and the ternary pass.
  - RoPE even/odd pairs contiguous via host weight-row permutation.
"""
import numpy as np
from contextlib import ExitStack

import concourse.bass as bass
import concourse.bacc as bacc
import concourse.mybir as mybir
import concourse.tile as tile
from concourse.bass_utils import run_bass_kernel_spmd
from concourse.masks import make_upper_triangular

B, S, D = 2, 2048, 2048
H, KH = 16, 4
HD = D // H          # 128
KVD = KH * HD        # 512
NB = S // 128        # 16 token blocks
SQ = S // 4          # 512 tokens per quarter
NBQ = NB // 4        # 4 blocks per quarter
EPS = 1e-8
MAGIC = float(1.5 * 2 ** 23)
ATANH05 = 0.5493061443340549      # arctanh(0.5)
F32 = mybir.dt.float32
BF16 = mybir.dt.bfloat16
AX = mybir.AxisListType
OP = mybir.AluOpType
AF = mybir.ActivationFunctionType

_cache = {}


def build(causal: bool, local_cc: bool = False):
    nc = bacc.Bacc()
    x_d = nc.dram_tensor("x", [SQ, D], F32, kind="ExternalInput")
    wq_d = nc.dram_tensor("wq", [D, KVD], F32, kind="ExternalInput")   # sel+perm+T
    wk_d = nc.dram_tensor("wk", [D, HD], F32, kind="ExternalInput")    # perm+T
    wv_d = nc.dram_tensor("wv", [D, HD], F32, kind="ExternalInput")    # T
    wo_d = nc.dram_tensor("wo", [KVD, D], F32, kind="ExternalInput")   # w_o.T full
    cos_d = nc.dram_tensor("cos", [S, HD // 2], F32, kind="ExternalInput")
    sin_d = nc.dram_tensor("sin", [S, HD // 2], F32, kind="ExternalInput")
    qsel_d = nc.dram_tensor("qsel", [128, 2], F32, kind="ExternalInput")
    y_d = nc.dram_tensor("y", [SQ, D], F32, kind="ExternalOutput")
    st_in = nc.dram_tensor("st_in", [1, 4], F32)
    st_out = nc.dram_tensor("st_out", [1, 4], F32, addr_space="Shared")
    agx_in = nc.dram_tensor("agx_in", [128, NB, SQ], BF16)
    agx_out = nc.dram_tensor("agx_out", [4, 128, NB, SQ], BF16)
    agd_in = nc.dram_tensor("agd_in", [128, NBQ], F32)
    agd_out = nc.dram_tensor("agd_out", [4, 128, NBQ], F32)
    cc_in = nc.dram_tensor("cc_in", [4, 8, 128, HD], BF16)
    cc_out = nc.dram_tensor("cc_out", [4, 8, 128, HD], BF16)
    AG_GROUPS = [[0, 1, 2, 3], [4, 5, 6, 7]]

    with tile.TileContext(nc) as tc, ExitStack() as ctx:
        cpool = ctx.enter_context(tc.tile_pool(name="const", bufs=1))
        sm = ctx.enter_context(tc.tile_pool(name="sm", bufs=1))
        wint = ctx.enter_context(tc.tile_pool(name="wint", bufs=1))

        # ---------- constants ----------
        tri01 = cpool.tile([128, 128], BF16, tag="tri01")
        if causal:
            make_upper_triangular(nc, tri01[:], val=1.0, diag=True)
        else:
            nc.gpsimd.memset(tri01[:], 1.0)
        ones_c = cpool.tile([128, 1], F32, tag="onc")
        nc.any.memset(ones_c[:], 1.0)
        ones_r = cpool.tile([1, 128], F32, tag="onr")
        nc.any.memset(ones_r[:], 1.0)
        inv_n = cpool.tile([128, 4], F32, tag="invn")
        for j, numel in enumerate([D * D, KVD * D, KVD * D, D * KVD]):
            nc.any.memset(inv_n[:, j:j + 1], 1.0 / (2.0 * numel))
        # cos/sin duplicated for batched q|k rope: [128, NB, 2, 64]
        cosd = cpool.tile([128, NB, 2, HD // 2], BF16, tag="cosd")
        sind = cpool.tile([128, NB, 2, HD // 2], BF16, tag="sind")
        qsel = cpool.tile([128, 2], F32, tag="qsel")

        # persistent small tiles
        deq_all = sm.tile([128, NB], F32, tag="deq_all")
        dq_all = sm.tile([128, NB], F32, tag="dq_all")
        dk_all = sm.tile([128, NB], F32, tag="dk_all")
        dv_all = sm.tile([128, NB], F32, tag="dv_all")
        partials = sm.tile([128, 10], F32, tag="partials")
        ptot = sm.tile([128, 4], F32, tag="ptot")
        st_sb = sm.tile([1, 4], F32, tag="st_sb")
        st2_sb = sm.tile([1, 4], F32, tag="st2_sb")
        totals = sm.tile([128, 4], F32, tag="totals")
        s4 = sm.tile([128, 4], F32, tag="s4")
        thr4 = sm.tile([128, 4], F32, tag="thr4")
        nthr4 = sm.tile([128, 4], F32, tag="nthr4")
        a4 = sm.tile([128, 4], F32, tag="a4")
        aq128 = sm.tile([128, 1], F32, tag="aq128")

        # ternary weights (persistent)
        wqkv = wint.tile([128, NB, 3 * HD], BF16, tag="wqkv")     # [dchunk, j, q|k|v]

        # attention-hot tiles in a pool that never overlaps the weight-hold
        # space (avoids WAR stalls on pool-space reuse)
        egbig = ctx.enter_context(tc.tile_pool(name="egbig", bufs=1))
        qkT = egbig.tile([128, NB, 2, 128], BF16, tag="qkT")      # [d, i, q|k, tok]
        v_aug = egbig.tile([128, NB, HD + 1], BF16, tag="vaug")   # [tok, i, d|1]
        nc.any.memset(v_aug[:, :, HD:HD + 1], 1.0)

        # ---------- weight chunks: streamed once, held in f32 ----------
        whold_ctx = ExitStack()
        whold = whold_ctx.enter_context(tc.tile_pool(name="whold", bufs=1))
        wch = [whold.tile([128, D], F32, tag=f"wch{i}", name=f"wch{i}")
               for i in range(6)]
        W_SRC = (
            [(wq_d.ap()[c * 512:(c + 1) * 512, :]
              .rearrange("(c p) f -> p c f", p=128), 4) for c in range(4)]
            + [(wk_d.ap().rearrange("(c p) f -> p c f", p=128), NB)]
            + [(wv_d.ap().rearrange("(c p) f -> p c f", p=128), NB)]
            + [(wo_d.ap()[c * 128:(c + 1) * 128, :], 1) for c in range(4)]
        )

        # ================= x phase: stats + int8 quant (own quarter)
        # x loads are issued FIRST (the x-quant chain is the early critical
        # path feeding the AllGather); weight streams queue behind them.
        xph_ctx = ExitStack()
        xph = xph_ctx.enter_context(tc.tile_pool(name="xph", bufs=1))
        xq_loc = xph.tile([128, NB, SQ], BF16, tag="xqloc")       # own slab
        xbs = [xph.tile([128, D], F32, tag="xb", bufs=4, name="xb")
               for bi in range(NBQ)]
        qbs = [xph.tile([128, D], BF16, tag="qb", bufs=4, name="qb")
               for bi in range(NBQ)]

        def emit_xload(bi):
            nc.sync.dma_start(xbs[bi][:], x_d[bi * 128:(bi + 1) * 128, :])

        def emit_wload(idx):
            src, nsub = W_SRC[idx]
            t = wch[idx]
            if nsub > 1:
                nc.sync.dma_start(t[:].rearrange("p (a b) -> p a b", a=nsub), src)
            else:
                nc.sync.dma_start(t[:], src)

        # interleave x and w streams: x feeds the quant->AllGather path, w
        # feeds the abs-mean -> AllReduce -> ternary path; both are critical
        for bi in range(NBQ):
            emit_xload(bi)
        for idx in range(6):
            emit_wload(idx)
        # rope tables + qsel are not needed until the QKV/out phases
        for h in range(2):
            nc.gpsimd.dma_start(cosd[:, :, h, :],
                                cos_d.ap().rearrange("(i p) f -> p i f", p=128))
            nc.gpsimd.dma_start(sind[:, :, h, :],
                                sin_d.ap().rearrange("(i p) f -> p i f", p=128))
        nc.gpsimd.dma_start(qsel[:], qsel_d[:])
        # batched stats for all 4 blocks: one [128, NBQ]-wide chain
        mx4 = xph.tile([128, NBQ], F32, tag="mx4")
        ssq4 = xph.tile([128, NBQ], F32, tag="ssq4")
        sq_scr = xph.tile([128, D], BF16, tag="sqscr")
        for bi in range(NBQ):
            nc.vector.tensor_reduce(mx4[:, bi:bi + 1], xbs[bi][:], axis=AX.X,
                                    op=OP.max, apply_absolute_value=True)
            nc.scalar.activation(sq_scr[:], xbs[bi][:], AF.Square,
                                 accum_out=ssq4[:, bi:bi + 1])
        mean4 = xph.tile([128, NBQ], F32, tag="mean4")
        nc.vector.tensor_scalar(mean4[:], ssq4[:], 1.0 / D, EPS,
                                op0=OP.mult, op1=OP.add)
        sd4 = xph.tile([128, NBQ], F32, tag="sd4")
        nc.scalar.activation(sd4[:], mean4[:], AF.Sqrt)
        r4 = xph.tile([128, NBQ], F32, tag="r4")
        nc.vector.reciprocal(r4[:], sd4[:])
        nt4a = xph.tile([128, NBQ], F32, tag="nt4a")
        nc.vector.tensor_tensor(nt4a[:], r4[:], r4[:], op=OP.mult)
        nc.vector.tensor_tensor(nt4a[:], nt4a[:], mean4[:], op=OP.mult)
        nc.vector.tensor_scalar(nt4a[:], nt4a[:], -0.5, 1.5, op0=OP.mult, op1=OP.add)
        nc.vector.tensor_tensor(r4[:], r4[:], nt4a[:], op=OP.mult)
        m4 = xph.tile([128, NBQ], F32, tag="m4")
        nc.vector.tensor_tensor(m4[:], r4[:], mx4[:], op=OP.mult)
        nc.vector.tensor_scalar(m4[:], m4[:], 1e-4, None, op0=OP.max)
        scl4 = xph.tile([128, NBQ], F32, tag="scl4")
        nc.vector.reciprocal(scl4[:], m4[:])
        nt4b = xph.tile([128, NBQ], F32, tag="nt4b")
        nc.vector.tensor_tensor(nt4b[:], m4[:], scl4[:], op=OP.mult)
        nc.vector.tensor_scalar(nt4b[:], nt4b[:], -1.0, 2.0, op0=OP.mult, op1=OP.add)
        nc.vector.tensor_tensor(scl4[:], scl4[:], nt4b[:], op=OP.mult)
        nc.vector.tensor_scalar(scl4[:], scl4[:], 127.0, None, op0=OP.mult)
        nc.vector.tensor_scalar(deq_all[:, 0:NBQ], m4[:], 1.0 / 127.0, None,
                                op0=OP.mult)
        smul4 = xph.tile([128, NBQ], F32, tag="smul4")
        nc.vector.tensor_tensor(smul4[:], r4[:], scl4[:], op=OP.mult)
        for bi in range(NBQ):
            # xb = xb*smul + MAGIC (DVE); qb = xb - MAGIC -> bf16 (Pool/DVE)
            nc.vector.tensor_scalar(xbs[bi][:], xbs[bi][:],
                                    smul4[:, bi:bi + 1], MAGIC,
                                    op0=OP.mult, op1=OP.add)
            nc.gpsimd.tensor_scalar(qbs[bi][:], xbs[bi][:], MAGIC, None,
                                    op0=OP.subtract)

        # ---------- weights pass 1: mean(|w|) partial sums
        # odd chunks on Act; even chunks on DVE (emitted after the x ops)
        scr = sq_scr[:]
        for idx in (1, 3, 5):
            nc.scalar.activation(scr, wch[idx][:], AF.Abs,
                                 accum_out=partials[:, idx:idx + 1])
        for idx in (0, 2, 4):
            nc.vector.tensor_reduce(partials[:, idx:idx + 1], wch[idx][:],
                                    axis=AX.X, op=OP.add,
                                    apply_absolute_value=True)

        # x transposes (SP queue, after the streams) + AllGather sends
        for bi in range(NBQ):
            nc.sync.dma_start_transpose(
                xq_loc[:, :, bi * 128:(bi + 1) * 128], qbs[bi][:])
        for j0 in range(0, NB, 4):
            nc.gpsimd.dma_start(agx_in.ap()[:, j0:j0 + 4, :],
                                xq_loc[:, j0:j0 + 4, :])
        nc.gpsimd.dma_start(agd_in.ap(), deq_all[:, 0:NBQ])
        if not local_cc:
            nc.gpsimd.collective_compute(
                "AllGather", OP.bypass, replica_groups=AG_GROUPS,
                ins=[agx_in.ap().opt()], outs=[agx_out.ap().opt()])
            nc.gpsimd.collective_compute(
                "AllGather", OP.bypass, replica_groups=AG_GROUPS,
                ins=[agd_in.ap().opt()], outs=[agd_out.ap().opt()])
        # wo chunks: stream through rotating bufs for the abs pass only
        # (re-streamed for their ternary during attention)
        for k, idx in enumerate((6, 7, 8, 9)):
            srcap, nsub = W_SRC[idx]
            t = whold.tile([128, D], F32, tag="wstr", bufs=2, name="wstr")
            nc.sync.dma_start(t[:], srcap)
            if k % 2 == 0:
                nc.vector.tensor_reduce(partials[:, idx:idx + 1], t[:],
                                        axis=AX.X, op=OP.add,
                                        apply_absolute_value=True)
            else:
                nc.scalar.activation(scr, t[:], AF.Abs,
                                     accum_out=partials[:, idx:idx + 1])
        xph_ctx.close()

        # ---------- global mean(|w|): qkv part first (gates ternary/QKV)
        nc.vector.tensor_reduce(ptot[:, 0:1], partials[:, 0:4], axis=AX.X, op=OP.add)
        nc.vector.tensor_copy(ptot[:, 1:2], partials[:, 4:5])
        nc.vector.tensor_copy(ptot[:, 2:3], partials[:, 5:6])
        nc.vector.tensor_reduce(ptot[:, 3:4], partials[:, 6:10], axis=AX.X, op=OP.add)
        nc.vector.tensor_scalar(ptot[:, 3:4], ptot[:, 3:4], 0.25, None, op0=OP.mult)
        def emit_streduce(sl, tag):
            with tc.tile_pool(name=f"ps1{tag}", bufs=1, space="PSUM") as ps1:
                w = sl.stop - sl.start
                pcw = ps1.tile([1, 4], F32, tag="mm1a", name="pcw")
                nc.tensor.matmul(pcw[:, 0:w], ones_c[:], ptot[:, sl],
                                 start=True, stop=True)
                nc.vector.tensor_copy(st_sb[:, sl], pcw[:, 0:w])
                nc.sync.dma_start(st_in[0:1, sl], st_sb[:, sl])
                if local_cc:
                    # local stand-in: read back what we wrote (the collective
                    # transport itself runs on the collective cores)
                    nc.sync.dma_start(st2_sb[:, sl], st_in[0:1, sl])
                else:
                    nc.gpsimd.collective_compute(
                        "AllReduce", OP.add, replica_groups=[list(range(8))],
                        ins=[st_in.ap()[0:1, sl].opt()],
                        outs=[st_out.ap()[0:1, sl].opt()])
                    nc.sync.dma_start(st2_sb[:, sl], st_out[0:1, sl])
                bcw = ps1.tile([128, 4], F32, tag="mm1b", name="bcw")
                nc.tensor.matmul(bcw[:, 0:w], ones_r[:], st2_sb[:, sl],
                                 start=True, stop=True)
                nc.vector.tensor_copy(totals[:, sl], bcw[:, 0:w])
            # s = totals * inv_n ; thr = ATANH05*(s+EPS) ; a = arctanh(s)
            nc.vector.tensor_tensor(s4[:, sl], totals[:, sl], inv_n[:, sl],
                                    op=OP.mult)
            nc.vector.tensor_scalar(thr4[:, sl], s4[:, sl], EPS, ATANH05,
                                    op0=OP.add, op1=OP.mult)
            nc.vector.tensor_scalar(nthr4[:, sl], thr4[:, sl], -1.0, None,
                                    op0=OP.mult)
            num = sm.tile([128, 4], F32, tag=f"num{tag}", name="num")
            den = sm.tile([128, 4], F32, tag=f"den{tag}", name="den")
            rat = sm.tile([128, 4], F32, tag=f"rat{tag}", name="rat")
            nc.vector.tensor_scalar(num[:, 0:w], s4[:, sl], 1.0, None, op0=OP.add)
            nc.vector.tensor_scalar(den[:, 0:w], s4[:, sl], -1.0, 1.0,
                                    op0=OP.mult, op1=OP.add)
            nc.vector.reciprocal(rat[:, 0:w], den[:, 0:w])
            ratn = sm.tile([128, 4], F32, tag=f"ratn{tag}", name="ratn")
            nc.vector.tensor_tensor(ratn[:, 0:w], den[:, 0:w], rat[:, 0:w], op=OP.mult)
            nc.vector.tensor_scalar(ratn[:, 0:w], ratn[:, 0:w], -1.0, 2.0,
                                    op0=OP.mult, op1=OP.add)
            nc.vector.tensor_tensor(rat[:, 0:w], rat[:, 0:w], ratn[:, 0:w], op=OP.mult)
            nc.vector.tensor_tensor(rat[:, 0:w], rat[:, 0:w], num[:, 0:w], op=OP.mult)
            lnr = sm.tile([128, 4], F32, tag=f"lnr{tag}", name="lnr")
            nc.scalar.activation(lnr[:, 0:w], rat[:, 0:w], AF.Ln)
            nc.vector.tensor_scalar(a4[:, sl], lnr[:, 0:w], 0.5, None, op0=OP.mult)

        nc.vector.tensor_scalar(aq128[:], a4[:, 0:1], 1.0 / 128.0, None, op0=OP.mult)

        emit_streduce(slice(0, 3), "q")

        # ---------- pass 2: ternary = 0.5*Sign(w+thr) + ((w>thr) - 0.5)
        with tc.tile_pool(name="tern", bufs=2) as ternp:
            def ternary(idx, thr_col, out_ap):
                t = wch[idx]
                g05 = ternp.tile([128, D], BF16, tag="g05", name="g05")
                nc.vector.tensor_scalar(g05[:], t[:],
                                        thr4[:, thr_col:thr_col + 1], 0.5,
                                        op0=OP.is_gt, op1=OP.subtract)
                sgn = ternp.tile([128, D], BF16, tag="sgn", name="sgn")
                nc.scalar.activation(sgn[:], t[:], AF.Sign,
                                     bias=thr4[:, thr_col:thr_col + 1])
                nc.vector.scalar_tensor_tensor(out_ap, sgn[:], 0.5, g05[:],
                                               op0=OP.mult, op1=OP.add)

            # wk, wv first (every QKV block needs them), then wq chunks
            ternary(4, 1, wqkv[:, :, HD:2 * HD])
            ternary(5, 2, wqkv[:, :, 2 * HD:3 * HD])
            for c in range(4):
                tq = ternp.tile([128, 4, KVD], BF16, tag="tq")
                ternary(c, 0, tq[:].rearrange("p a b -> p (a b)"))
                e1 = ternp.tile([128, 4, HD], BF16, tag="e1")
                nc.vector.tensor_tensor(e1[:], tq[:, :, 0:HD],
                                        tq[:, :, HD:2 * HD], op=OP.add)
                e2 = ternp.tile([128, 4, HD], BF16, tag="e2")
                nc.vector.tensor_tensor(e2[:], tq[:, :, 2 * HD:3 * HD],
                                        tq[:, :, 3 * HD:4 * HD], op=OP.add)
                nc.vector.tensor_tensor(wqkv[:, c * 4:(c + 1) * 4, 0:HD],
                                        e1[:], e2[:], op=OP.add)
        # wo's own (small) AllReduce: only gates wo-ternary / out-projection
        emit_streduce(slice(3, 4), "o")
        whold_ctx.close()

        # ================= QKV + rope + attention (software-pipelined)
        with tc.tile_pool(name="gbig", bufs=1) as gbig, \
             tc.tile_pool(name="qkv", bufs=1) as qkv, \
             tc.tile_pool(name="attn", bufs=1) as attn:
            psattn_ctx = ExitStack()
            psst = psattn_ctx.enter_context(
                tc.tile_pool(name="psst", bufs=2, space="PSUM"))
            pspo = psattn_ctx.enter_context(
                tc.tile_pool(name="pspo", bufs=2, space="PSUM"))
            psq_ctx = ExitStack()
            psq = psq_ctx.enter_context(tc.tile_pool(name="psq", bufs=2, space="PSUM"))

            wo_i = gbig.tile([128, 4, D], BF16, tag="wo_i")         # [dchunk, c, D]
            obq = gbig.tile([128, 4, 4, HD], BF16, tag="obq")       # [tok, q, r, d]

            def emit_qkv(i, xqs):
                lo = (i % NBQ) * 128
                pq = psq.tile([128, 3 * HD], F32, tag="pq")
                for j in range(NB):
                    nc.tensor.matmul(pq[:], xqs[:, j, lo:lo + 128],
                                     wqkv[:, j, :], start=(j == 0), stop=(j == NB - 1))
                # evict with dequant scales: qkn [128, 2, 128] bf16
                qkn = egbig.tile([128, 2, HD], BF16, tag="qkn", bufs=2)
                nc.vector.tensor_scalar(qkn[:, 0, :], pq[:, 0:HD],
                                        dq_all[:, i:i + 1], None, op0=OP.mult)
                nc.vector.tensor_scalar(qkn[:, 1, :], pq[:, HD:2 * HD],
                                        dk_all[:, i:i + 1], None, op0=OP.mult)
                nc.scalar.activation(v_aug[:, i, 0:HD], pq[:, 2 * HD:3 * HD],
                                     AF.Copy, scale=dv_all[:, i:i + 1])
                # rope on q|k batched halves (DVE, bf16 2x)
                hh = HD // 2
                x0 = qkn[:, :, 0:hh]
                x1 = qkn[:, :, hh:HD]
                ci = cosd[:, i, :, :]
                si = sind[:, i, :, :]
                t1 = egbig.tile([128, 2, hh], BF16, tag="t1", bufs=2)
                t2 = egbig.tile([128, 2, hh], BF16, tag="t2", bufs=2)
                t3 = egbig.tile([128, 2, hh], BF16, tag="t3", bufs=2)
                t4 = egbig.tile([128, 2, hh], BF16, tag="t4", bufs=2)
                qkr = egbig.tile([128, 2, HD], BF16, tag="qkr", bufs=2)
                nc.vector.tensor_tensor(t1[:], x0, ci, op=OP.mult)
                nc.vector.tensor_tensor(t2[:], x1, si, op=OP.mult)
                nc.vector.tensor_tensor(qkr[:, :, 0:hh], t1[:], t2[:], op=OP.subtract)
                nc.vector.tensor_tensor(t3[:], x0, si, op=OP.mult)
                nc.vector.tensor_tensor(t4[:], x1, ci, op=OP.mult)
                nc.vector.tensor_tensor(qkr[:, :, hh:HD], t3[:], t4[:], op=OP.add)
                # transpose q|k -> qkT[:, i, :, :]
                nc.scalar.dma_start_transpose(qkT[:, i, :, :], qkr[:])

            def emit_scores(i):
                # ST[key, tok] in halves of <=8 key blocks; exp -> PT bf16
                nk = (i + 1) if causal else NB
                PT = attn.tile([128, NB, 128], BF16, tag="PT", bufs=2)
                for h in range((nk + 7) // 8):
                    k0 = h * 8
                    k1 = min(nk, k0 + 8)
                    stp = psst.tile([128, 8, 128], F32, tag="st")
                    for kb in range(k0, k1):
                        nc.tensor.matmul(stp[:, kb - k0, :], qkT[:, kb, 1, :],
                                         qkT[:, i, 0, :], start=True, stop=True)
                    nc.scalar.activation(
                        PT[:, k0:k1, :].rearrange("p a b -> p (a b)"),
                        stp[:, 0:k1 - k0, :].rearrange("p a b -> p (a b)"), AF.Exp)
                if causal:
                    nc.vector.tensor_tensor(PT[:, nk - 1, :], PT[:, nk - 1, :],
                                            tri01[:], op=OP.mult)
                return PT, nk

            def emit_pv(i, PT, nk):
                po = pspo.tile([128, HD + 1], F32, tag="po")
                for kb in range(nk):
                    nc.tensor.matmul(po[:], PT[:, kb, :], v_aug[:, kb, :],
                                     start=(kb == 0), stop=(kb == nk - 1))
                rz = attn.tile([128, 1], F32, tag="rz", bufs=2)
                nc.vector.reciprocal(rz[:], po[:, HD:HD + 1])
                nt = attn.tile([128, 1], F32, tag="nt", bufs=2)
                nc.vector.tensor_tensor(nt[:], po[:, HD:HD + 1], rz[:], op=OP.mult)
                nc.vector.tensor_scalar(nt[:], nt[:], -1.0, 2.0, op0=OP.mult, op1=OP.add)
                nc.vector.tensor_tensor(rz[:], rz[:], nt[:], op=OP.mult)
                nc.vector.tensor_scalar(obq[:, i // 4, i % 4, :], po[:, 0:HD],
                                        rz[:], None, op0=OP.mult)
                q, rr = i // 4, i % 4
                nc.gpsimd.dma_start(cc_in.ap()[rr, q], obq[:, q, rr, :])
                nc.gpsimd.dma_start(cc_in.ap()[rr, q + 4], obq[:, q, rr, :])

            # per-slot: gather-read xqT slab + deq, dequant scales, QKV blocks
            att = [0, None]
            for g in range(4):
                if local_cc:
                    if g == 0:
                        for j0 in range(0, NB, 4):
                            nc.sync.dma_start(agx_out.ap()[0][:, j0:j0 + 4, :],
                                              agx_in.ap()[:, j0:j0 + 4, :])
                    nc.sync.dma_start(agd_out.ap()[g], agd_in.ap())
                xqs = egbig.tile([128, NB, SQ], BF16, tag="xqs", bufs=2, name="xqs")
                for j0 in range(0, NB, 4):
                    nc.sync.dma_start(xqs[:, j0:j0 + 4, :],
                                      agx_out.ap()[g][:, j0:j0 + 4, :])
                nc.sync.dma_start(deq_all[:, g * NBQ:(g + 1) * NBQ], agd_out.ap()[g])
                sl = slice(g * NBQ, (g + 1) * NBQ)
                nc.vector.tensor_scalar(dq_all[:, sl], deq_all[:, sl],
                                        aq128[:, 0:1], None, op0=OP.mult)
                nc.vector.tensor_scalar(dk_all[:, sl], deq_all[:, sl],
                                        a4[:, 1:2], None, op0=OP.mult)
                nc.vector.tensor_scalar(dv_all[:, sl], deq_all[:, sl],
                                        a4[:, 2:3], None, op0=OP.mult)
                for bi in range(NBQ):
                    emit_qkv(g * NBQ + bi, xqs)
                # attention for all blocks this slot unlocked (PE fills the
                # next slot's gather latency with ST/PV work)
                while att[0] < min((g + 1) * NBQ, 13):
                    i = att[0]
                    cur = emit_scores(i)
                    if att[1] is not None:
                        emit_pv(i - 1, *att[1])
                    att[0], att[1] = i + 1, cur
            psq_ctx.close()

            # wo ternary: re-stream chunks now (gather traffic has drained)
            with tc.tile_pool(name="wstr3", bufs=2) as wstr3:
                for c in range(4):
                    t2w = wstr3.tile([128, D], F32, tag="w2", name="w2")
                    nc.sync.dma_start(t2w[:], wo_d.ap()[c * 128:(c + 1) * 128, :])
                    gw = wstr3.tile([128, D], BF16, tag="gw", name="gw")
                    nc.gpsimd.tensor_scalar(gw[:], t2w[:], thr4[:, 3:4],
                                            None, op0=OP.is_gt)
                    lw = wstr3.tile([128, D], BF16, tag="lw", name="lw")
                    nc.gpsimd.tensor_scalar(lw[:], t2w[:], nthr4[:, 3:4],
                                            None, op0=OP.is_lt)
                    nc.vector.tensor_tensor(wo_i[:, c, :], gw[:], lw[:],
                                            op=OP.subtract)

            # tail: finish attention blocks 12-15 interleaved with per-row
            # AllToAll exchanges and the output projection
            for tb in range(4):
                if 13 + tb <= 15:
                    cur = emit_scores(13 + tb)
                    emit_pv(12 + tb, *att[1])
                    att[1] = cur
                else:
                    emit_pv(15, *att[1])
                # exchange row-block tb (padded 8-way AllToAll)
                if local_cc:
                    nc.gpsimd.dma_start(cc_out.ap()[tb], cc_in.ap()[tb])
                else:
                    nc.gpsimd.collective_compute(
                        "AllToAll", OP.bypass, replica_groups=[list(range(8))],
                        ins=[cc_in.ap()[tb].opt()], outs=[cc_out.ap()[tb].opt()])
            psattn_ctx.close()
            outp_ctx = ExitStack()
            outp = outp_ctx.enter_context(tc.tile_pool(name="outp", bufs=1))
            psy = outp_ctx.enter_context(
                tc.tile_pool(name="psy", bufs=2, space="PSUM"))
            osc = outp.tile([128, KVD], BF16, tag="osc")

            # gather + combine all four row-blocks; batched [128,4] stats
            mxo = outp.tile([128, 4], F32, tag="mxo")
            sqo = outp.tile([128, 4], F32, tag="sqo")
            xos = []
            for tb in range(4):
                xo8 = outp.tile([128, 8, HD], BF16, tag="xo8", bufs=4, name="xo8")
                nc.gpsimd.dma_start(xo8[:], cc_out.ap()[tb]
                                    .rearrange("j p d -> p j d"))
                xoa = outp.tile([128, KVD], BF16, tag="xoa", bufs=2, name="xoa")
                nc.vector.tensor_scalar(
                    xoa[:], xo8[:, 0:4, :].rearrange("p a b -> p (a b)"),
                    qsel[:, 0:1], None, op0=OP.mult)
                xos.append(outp.tile([128, KVD], F32, tag="xo", bufs=4, name="xo"))
                nc.vector.scalar_tensor_tensor(
                    xos[tb][:], xo8[:, 4:8, :].rearrange("p a b -> p (a b)"),
                    qsel[:, 1:2], xoa[:], op0=OP.mult, op1=OP.add)
                nc.vector.tensor_reduce(mxo[:, tb:tb + 1], xos[tb][:], axis=AX.X,
                                        op=OP.max, apply_absolute_value=True)
                nc.scalar.activation(osc[:], xos[tb][:], AF.Square,
                                     accum_out=sqo[:, tb:tb + 1])
            meo = outp.tile([128, 4], F32, tag="meo")
            nc.vector.tensor_scalar(meo[:], sqo[:], 1.0 / KVD, EPS,
                                    op0=OP.mult, op1=OP.add)
            sdo = outp.tile([128, 4], F32, tag="sdo")
            nc.scalar.activation(sdo[:], meo[:], AF.Sqrt)
            ro = outp.tile([128, 4], F32, tag="ro")
            nc.vector.reciprocal(ro[:], sdo[:])
            nto = outp.tile([128, 4], F32, tag="nto")
            nc.vector.tensor_tensor(nto[:], ro[:], ro[:], op=OP.mult)
            nc.vector.tensor_tensor(nto[:], nto[:], meo[:], op=OP.mult)
            nc.vector.tensor_scalar(nto[:], nto[:], -0.5, 1.5, op0=OP.mult, op1=OP.add)
            nc.vector.tensor_tensor(ro[:], ro[:], nto[:], op=OP.mult)
            mo = outp.tile([128, 4], F32, tag="mo")
            nc.vector.tensor_tensor(mo[:], ro[:], mxo[:], op=OP.mult)
            nc.vector.tensor_scalar(mo[:], mo[:], 1e-4, None, op0=OP.max)
            sco = outp.tile([128, 4], F32, tag="sco")
            nc.vector.reciprocal(sco[:], mo[:])
            nto2 = outp.tile([128, 4], F32, tag="nto2")
            nc.vector.tensor_tensor(nto2[:], mo[:], sco[:], op=OP.mult)
            nc.vector.tensor_scalar(nto2[:], nto2[:], -1.0, 2.0, op0=OP.mult, op1=OP.add)
            nc.vector.tensor_tensor(sco[:], sco[:], nto2[:], op=OP.mult)
            nc.vector.tensor_scalar(sco[:], sco[:], 127.0, None, op0=OP.mult)
            dqo = outp.tile([128, 4], F32, tag="dqo")
            nc.vector.tensor_scalar(dqo[:], mo[:], 1.0 / 127.0, None, op0=OP.mult)
            nc.vector.tensor_scalar(dqo[:], dqo[:], a4[:, 3:4], None, op0=OP.mult)
            smo = outp.tile([128, 4], F32, tag="smo")
            nc.vector.tensor_tensor(smo[:], ro[:], sco[:], op=OP.mult)
            # quantize, transpose, project
            for tb in range(4):
                nc.vector.tensor_scalar(xos[tb][:], xos[tb][:],
                                        smo[:, tb:tb + 1], MAGIC,
                                        op0=OP.mult, op1=OP.add)
                qo = outp.tile([128, KVD], BF16, tag="qo", bufs=2, name="qo")
                nc.gpsimd.tensor_scalar(qo[:], xos[tb][:], MAGIC, None,
                                        op0=OP.subtract)
                xoT = outp.tile([128, 4, 128], BF16, tag="xoT", bufs=2, name="xoT")
                nc.scalar.dma_start_transpose(xoT[:], qo[:])
                y_sb = outp.tile([128, D], F32, tag="ysb", bufs=2, name="ysb")
                for oc in range(4):
                    py = psy.tile([128, 512], F32, tag="py")
                    for jc in range(4):
                        nc.tensor.matmul(py[:], xoT[:, jc, :],
                                         wo_i[:, jc, oc * 512:(oc + 1) * 512],
                                         start=(jc == 0), stop=(jc == 3))
                    nc.scalar.activation(y_sb[:, oc * 512:(oc + 1) * 512], py[:],
                                         AF.Copy, scale=dqo[:, tb:tb + 1])
                nc.gpsimd.dma_start(y_d[tb * 128:(tb + 1) * 128, :], y_sb[:])
            outp_ctx.close()
    nc.compile()
    return nc


def _rope_perm():
    p = np.empty(HD, np.int64)
    p[:HD // 2] = np.arange(0, HD, 2)
    p[HD // 2:] = np.arange(1, HD, 2)
    return p


def qsel_host(b):
    q = np.zeros((128, 2), np.float32)
    q[:, b] = 1.0
    return q


def _prep_inputs(inputs):
    x = np.ascontiguousarray(np.asarray(inputs["x"], np.float32))
    w_q = np.asarray(inputs["w_q"], np.float32)
    w_k = np.asarray(inputs["w_k"], np.float32)
    w_v = np.asarray(inputs["w_v"], np.float32)
    w_o = np.asarray(inputs["w_o"], np.float32)
    cos = np.ascontiguousarray(np.asarray(inputs["freq_cos"], np.float32))
    sin = np.ascontiguousarray(np.asarray(inputs["freq_sin"], np.float32))
    perm = _rope_perm()
    woT = np.ascontiguousarray(w_o.T)                      # [KVD, D]
    in_maps = []
    for r in range(8):
        b, kh = r // 4, r % 4
        heads = [g * KH + kh for g in range(4)]
        wq_sel = w_q.reshape(H, HD, D)[heads][:, perm, :]  # [4,128,D]
        wqT = np.ascontiguousarray(wq_sel.reshape(4 * HD, D).T)   # [D, 512]
        wkT = np.ascontiguousarray(w_k[kh * HD:(kh + 1) * HD][perm].T)  # [D,128]
        wvT = np.ascontiguousarray(w_v[kh * HD:(kh + 1) * HD].T)        # [D,128]
        in_maps.append({
            "x": np.ascontiguousarray(x[b, kh * SQ:(kh + 1) * SQ, :]),
            "wq": wqT, "wk": wkT, "wv": wvT, "wo": woT,
            "cos": cos, "sin": sin,
            "qsel": qsel_host(b),
        })
    return in_maps


def _gains_trivial(inputs):
    return all(np.all(np.asarray(inputs[g]) == 1.0)
               for g in ("g_q", "g_k", "g_v", "g_o"))


def _numpy_fallback(inputs):
    """Faithful numpy reimplementation (slow); used only for unexpected configs."""
    x = np.asarray(inputs["x"], np.float32)
    cos, sin = (np.asarray(inputs[k], np.float32) for k in ("freq_cos", "freq_sin"))
    causal = int(np.asarray(inputs["causal"]))

    def rms(t, g):
        n = t * (1.0 / np.sqrt(np.mean(t * t, -1, keepdims=True, dtype=np.float32) + EPS))
        return (g * n).astype(np.float32)

    def actq(t):
        scale = 127.0 / np.clip(np.max(np.abs(t), -1, keepdims=True), 1e-4, None)
        q = np.round(t * scale)
        return np.clip(q, -128, 127) / scale

    def ternq(w):
        s = np.mean(np.abs(w), dtype=np.float32)
        return np.round(np.tanh(w / (s + EPS))) * np.arctanh(s)

    def lin(t, w, g):
        return actq(rms(t, g)).astype(np.float32) @ ternq(np.asarray(w, np.float32)).T

    Bb, Ss, Dd = x.shape
    q = lin(x, inputs["w_q"], np.asarray(inputs["g_q"], np.float32)).reshape(Bb, Ss, H, HD)
    k = lin(x, inputs["w_k"], np.asarray(inputs["g_k"], np.float32)).reshape(Bb, Ss, KH, HD)
    v = lin(x, inputs["w_v"], np.asarray(inputs["g_v"], np.float32)).reshape(Bb, Ss, KH, HD)

    def rope(t):
        t2 = t.reshape(*t.shape[:-1], -1, 2)
        c = cos[None, :, None, :]
        s_ = sin[None, :, None, :]
        o0 = t2[..., 0] * c - t2[..., 1] * s_
        o1 = t2[..., 0] * s_ + t2[..., 1] * c
        return np.stack([o0, o1], -1).reshape(t.shape).astype(np.float32)

    q, k = rope(q), rope(k)
    scale = np.float32(HD ** 0.5)
    q = q.transpose(0, 2, 1, 3) / scale
    k = k.transpose(0, 2, 1, 3)
    v = v.transpose(0, 2, 1, 3)
    qg = q.reshape(Bb, 4, KH, Ss, HD).sum(1)
    sc = np.einsum("bhnd,bhsd->bhns", qg, k).astype(np.float32)
    if causal:
        mask = np.tril(np.ones((Ss, Ss), bool))
        sc = np.where(mask[None, None], sc, np.float32(np.finfo(np.float32).min))
    sc = sc / scale
    sc = sc - sc.max(-1, keepdims=True)
    p = np.exp(sc)
    p /= p.sum(-1, keepdims=True)
    out = np.einsum("bhns,bhsd->bnhd", p, v).reshape(Bb, Ss, KVD)
    return lin(out, inputs["w_o"], np.asarray(inputs["g_o"], np.float32))


def kernel(**inputs):
    x = np.asarray(inputs["x"])
    if x.shape != (B, S, D) or not _gains_trivial(inputs):
        return _numpy_fallback(inputs)
    causal = bool(int(np.asarray(inputs["causal"])))
    key = ("bitattn2", causal)
    if key not in _cache:
        _cache[key] = build(causal)
    nc = _cache[key]
    in_maps = _prep_inputs(inputs)
    res = run_bass_kernel_spmd(nc, in_maps, core_ids=list(range(8)))
    y = np.empty((B, S, D), np.float32)
    for r in range(8):
        b, qq = r // 4, r % 4
        y[b, qq * SQ:(qq + 1) * SQ, :] = res.results[r]["y"]
    return y


if __name__ == "__main__":
    data = np.load("/tmp/inputs.npz")
    inputs = {k: data[k] for k in data.files}
    out = kernel(**inputs)
    exp = np.load("/tmp/expected.npy")
    err = np.linalg.norm(out - exp) / np.linalg.norm(exp)
    print("Relative error:", err)
